# revision 17
# baseline (speedup 1.0000x reference)
"""Multi-Head Latent Attention kernel for 8 Trainium2 NeuronCores — v2.

Sharding (unchanged from v1): data-parallel over (batch x strided query-block
sets). core c: batch b = c // 4, idx = c % 4; own query blocks g = idx + 4*j,
j in 0..3. Each core redundantly computes latent/K/V for its batch (cross-core
collectives are far slower than recompute) -> zero cross-core communication.
Causality via padded core-uniform block structure (NK(j) = 4j+4 key blocks)
plus per-core {0,1} multiplicative masks applied after exp.

v2 changes (all about PE occupancy — PE work is ~360us, v1 ran 454us):
 - startup: e-granular DMA + e-major latent loop; first matmul at ~1.5us
   instead of ~20us (no HAM warm-up loop needed).
 - 8-bank PSUM rotation in all projection phases (kills the per-iteration
   evacuation stalls k-phase had with 4 banks).
 - v computed in its own phase (frees latT/wuv before attention -> SBUF room
   for resident+streamed w_out tiles; w_out traffic 23MB vs 32MB).
 - attention emission is software-pipelined: per head emit
   [scores g0,g1] [out-proj(h-2)] [scores g2..] [ctx g0..] [normalize chain]
   so the PE never sits in-order-blocked on the ACT exp latency (~611ns/tile).
 - ACT does exp ONLY during attention (normalize runs on DVE); out-proj
   deferral is 2 heads (the v1 1-head deferral stalled on the transpose
   chain); bias is added on the host (saves 16 N=512 bias matmuls + SBUF).
"""

import math

import numpy as np
import ml_dtypes

import concourse.bacc as bacc
import concourse.mybir as mybir
import concourse.tile as tile

bf16 = ml_dtypes.bfloat16

EMB = 2048
HEADS = 16
D = 128          # head dim
L = 512          # latent dim
B, S = 2, 2048
NCORES = 8

EC = EMB // 128  # 16 e-chunks
LC = L // 128    # 4 l-chunks
QB = 4           # own q-blocks per core
NQ = QB * 128    # 512 own queries
SC = S // 512    # 4 s-chunks of 512
ST = S // 128    # 16 s-tiles of 128
NGRP = 4

WO_RES = 6       # w_out heads resident in SBUF (loaded once, in phase C)

_CACHE = {}


def build_program():
    nc = bacc.Bacc("TRN2", target_bir_lowering=False, debug=False)
    dt = mybir.dt

    xT = nc.dram_tensor("xT", [EMB, S], dt.bfloat16, kind="ExternalInput")
    xTq = nc.dram_tensor("xTq", [EMB, NQ], dt.bfloat16, kind="ExternalInput")
    wdT = nc.dram_tensor("wdT", [EMB, L], dt.bfloat16, kind="ExternalInput")
    wukT = nc.dram_tensor("wukT", [L, EMB], dt.bfloat16, kind="ExternalInput")
    wuvT = nc.dram_tensor("wuvT", [L, EMB], dt.bfloat16, kind="ExternalInput")
    # wq4[ftp, p, c*256]: f-tile pair ftp (256 f cols), partition-major so the
    # per-ftp DMA is 128 contiguous 8KB rows (512B rows stall the ring).
    wq4 = nc.dram_tensor("wq4", [EC // 2, 128, EC * 256], dt.bfloat16, kind="ExternalInput")
    woT = nc.dram_tensor("woT", [EMB, EMB], dt.bfloat16, kind="ExternalInput")
    masks = nc.dram_tensor("masks", [NGRP, 128, 512], dt.bfloat16, kind="ExternalInput")
    out = nc.dram_tensor("out", [NQ, EMB], dt.float32, kind="ExternalOutput")

    ident_t = nc.inline_tensor(np.eye(128, dtype=bf16), name="ident")

    scale = 1.0 / math.sqrt(D)
    import contextlib

    with tile.TileContext(nc) as tc, contextlib.ExitStack() as es:
        # ---- persistent (right-side) pools ----
        consts = es.enter_context(tc.tile_pool(name="consts", bufs=1, side="right"))
        p_qT = es.enter_context(tc.tile_pool(name="p_qT", bufs=1, side="right"))
        p_masks = es.enter_context(tc.tile_pool(name="p_masks", bufs=1, side="right"))

        ident = consts.tile([128, 128], dt.bfloat16)
        qT_sb = p_qT.tile([128, HEADS * NQ], dt.bfloat16)
        masks_sb = p_masks.tile([128, NGRP * 512], dt.bfloat16)

        # left-stack pools, LIFO: wo_res is bottom (lives longest), then latT,
        # wuv, wuk, wq, wd, xt (innermost, dies first)
        wores_cm = tc.tile_pool(name="p_wores", bufs=1)
        p_wores = wores_cm.__enter__()

        lat_cm = tc.tile_pool(name="p_lat", bufs=1)
        p_lat = lat_cm.__enter__()
        latT_sb = p_lat.tile([128, LC * S], dt.bfloat16)

        wuv_cm = tc.tile_pool(name="p_wuv", bufs=1)
        p_wuv = wuv_cm.__enter__()
        wuv_sb = p_wuv.tile([128, LC * EMB], dt.bfloat16)

        wuk_cm = tc.tile_pool(name="p_wuk", bufs=1)
        p_wuk = wuk_cm.__enter__()
        wuk_sb = p_wuk.tile([128, LC * EMB], dt.bfloat16)

        wq_cm = tc.tile_pool(name="p_wq", bufs=2)
        p_wq = wq_cm.__enter__()
        xtq_sb = p_wq.tile([128, EC * NQ], dt.bfloat16, tag="xtq", bufs=1)

        ps8_cm = tc.tile_pool(name="ps8", bufs=2, space="PSUM")
        ps8 = ps8_cm.__enter__()

        def evac(dst, src, which):
            if which % 2 == 0:
                nc.vector.tensor_copy(dst, src)
            else:
                nc.scalar.copy(dst, src)



        # ============ phase A: latentT[l, s] = wdT.T @ xT ============
        # e-granular DMA; e-major matmul loop so compute starts after the
        # first 256KB instead of after 6MB.
        wqs_tiles = {}
        with tc.tile_pool(name="p_wd", bufs=1) as p_wd, \
             tc.tile_pool(name="p_xt", bufs=2) as p_xt:
            wd_sb = p_wd.tile([128, EC * L], dt.bfloat16)
            for sc in range(SC):
                xt = p_xt.tile([128, EC * 512], dt.bfloat16, tag="xt")
                for e in range(EC):
                    if sc == 0:
                        nc.gpsimd.dma_start(
                            out=wd_sb[:, e * L:(e + 1) * L],
                            in_=wdT[e * 128:(e + 1) * 128, :])
                    eng = nc.sync if e % 2 == 0 else nc.gpsimd
                    eng.dma_start(
                        out=xt[:, e * 512:(e + 1) * 512],
                        in_=xT[e * 128:(e + 1) * 128, sc * 512:(sc + 1) * 512])
                # ring-paced prefetches for phases B/C
                if sc in (1, 2):
                    half = sc - 1
                    nc.sync.dma_start(
                        out=xtq_sb.rearrange("p (c q) -> p c q", c=EC)[
                            :, half * 8:(half + 1) * 8, :],
                        in_=xTq[half * 1024:(half + 1) * 1024, :].rearrange(
                            "(c p) q -> p c q", p=128))
                if sc in (2, 3):
                    i0 = sc - 2
                    wqs0 = p_wq.tile([128, EC * 256], dt.bfloat16,
                                     tag="wqs", bufs=3, name="wqs0")
                    nc.sync.dma_start(out=wqs0, in_=wq4[i0])
                    wqs_tiles[i0] = wqs0
                accs = [ps8.tile([128, 512], dt.float32, tag=f"a{lt}", name=f"lacc{lt}")
                        for lt in range(LC)]
                for e in range(EC):
                    for lt in range(LC):
                        nc.tensor.matmul(
                            accs[lt],
                            wd_sb[:, e * L + lt * 128: e * L + (lt + 1) * 128],
                            xt[:, e * 512:(e + 1) * 512],
                            start=(e == 0), stop=(e == EC - 1))
                for lt in range(LC):
                    evac(latT_sb[:, lt * S + sc * 512: lt * S + (sc + 1) * 512],
                         accs[lt], lt)

        # ============ phase B: qT[f, own q] = wq.T @ xTq ============
        for ftp in range(EC // 2):
            if ftp in (0, 2):
                half = ftp // 2
                nc.sync.dma_start(
                    out=wuk_sb.rearrange("p (c f) -> p c f", c=LC)[
                        :, half * 2:(half + 1) * 2, :],
                    in_=wukT[half * 256:(half + 1) * 256, :].rearrange(
                        "(c p) f -> p c f", p=128))
            if ftp + 2 < EC // 2:
                nxt = p_wq.tile([128, EC * 256], dt.bfloat16, tag="wqs",
                                bufs=3, name="wqsn")
                nc.sync.dma_start(out=nxt, in_=wq4[ftp + 2])
                wqs_tiles[ftp + 2] = nxt
            wqs = wqs_tiles.pop(ftp)
            for fi in range(2):
                ft = 2 * ftp + fi
                acc = ps8.tile([128, NQ], dt.float32, tag=f"a{ft % 4}", name="qacc")
                for e in range(EC):
                    nc.tensor.matmul(
                        acc,
                        wqs[:, e * 256 + fi * 128: e * 256 + (fi + 1) * 128],
                        xtq_sb[:, e * NQ:(e + 1) * NQ],
                        start=(e == 0), stop=(e == EC - 1))
                evac(qT_sb[:, ft * NQ:(ft + 1) * NQ], acc, ft)
        wq_cm.__exit__(None, None, None)

        # ============ phase C: kT[f, s] = wukT.T @ latentT ============
        p_kT = es.enter_context(tc.tile_pool(name="p_kT", bufs=1, side="right"))
        kT_sb = p_kT.tile([128, HEADS * S], dt.bfloat16)
        wo_res = {}
        for ft in range(EC):
            if ft in (2, 4):
                half = ft // 2 - 1
                nc.sync.dma_start(
                    out=wuv_sb.rearrange("p (c f) -> p c f", c=LC)[
                        :, half * 2:(half + 1) * 2, :],
                    in_=wuvT[half * 256:(half + 1) * 256, :].rearrange(
                        "(c p) f -> p c f", p=128))
            if ft == 6:
                nc.sync.dma_start(
                    out=masks_sb.rearrange("p (g q) -> p g q", g=NGRP),
                    in_=masks.rearrange("g p q -> p g q"))
            if ft == 8:
                nc.sync.dma_start(out=ident, in_=ident_t[:, :])
            if 10 <= ft < 10 + WO_RES:
                h = ft - 10
                wr = p_wores.tile([128, EMB], dt.bfloat16, tag=f"wr{h}")
                nc.sync.dma_start(out=wr, in_=woT[h * 128:(h + 1) * 128, :])
                wo_res[h] = wr
            kaccs = [ps8.tile([128, 512], dt.float32, tag=f"a{sc}", name=f"kacc{sc}")
                     for sc in range(SC)]
            for lc in range(LC):
                for sc in range(SC):
                    nc.tensor.matmul(
                        kaccs[sc],
                        wuk_sb[:, lc * EMB + ft * 128: lc * EMB + (ft + 1) * 128],
                        latT_sb[:, lc * S + sc * 512: lc * S + (sc + 1) * 512],
                        start=(lc == 0), stop=(lc == LC - 1))
            for sc in range(SC):
                evac(kT_sb[:, ft * S + sc * 512: ft * S + (sc + 1) * 512],
                     kaccs[sc], ft + sc)
        wuk_cm.__exit__(None, None, None)

        # ============ phase V: v[s, (h,129)] = latentT.T @ wuvT ============
        p_v = es.enter_context(tc.tile_pool(name="p_v", bufs=1, side="right"))
        v_sb = p_v.tile([128, ST * HEADS * (D + 1)], dt.bfloat16)
        nc.vector.memset(
            v_sb.rearrange("p (t w) -> p t w", w=D + 1)[:, :, D:D + 1], 1.0)
        for st in range(ST):
            for fc in range(SC):
                acc = ps8.tile([128, 512], dt.float32, tag=f"a{fc}", name="vacc")
                for lc in range(LC):
                    nc.tensor.matmul(
                        acc,
                        latT_sb[:, lc * S + st * 128: lc * S + (st + 1) * 128],
                        wuv_sb[:, lc * EMB + fc * 512: lc * EMB + (fc + 1) * 512],
                        start=(lc == 0), stop=(lc == LC - 1))
                base = st * HEADS * (D + 1) + fc * 4 * (D + 1)
                dst = v_sb[:, base: base + 4 * (D + 1)].rearrange(
                    "p (h w) -> p h w", h=4)[:, :, 0:D]
                src = acc.rearrange("p (h w) -> p h w", h=4)
                if st == ST - 1:
                    nc.scalar.copy(dst, src)
                else:
                    nc.vector.tensor_copy(dst, src)
        wuv_cm.__exit__(None, None, None)
        lat_cm.__exit__(None, None, None)
        ps8_cm.__exit__(None, None, None)

        # ============ attention + out-proj ============
        with tc.tile_pool(name="p_wos", bufs=4) as p_wos, \
             tc.tile_pool(name="p_attn", bufs=7) as p_attn, \
             tc.tile_pool(name="p_ctxn", bufs=3) as p_ctxn, \
             tc.tile_pool(name="p_ctxT", bufs=4) as p_ctxT, \
             tc.tile_pool(name="p_out", bufs=3) as p_out, \
             tc.tile_pool(name="p_rcp", bufs=4) as p_rcp, \
             tc.tile_pool(name="ps_oa", bufs=1, space="PSUM") as ps_oa, \
             tc.tile_pool(name="ps_sc", bufs=3, space="PSUM") as ps_sc, \
             tc.tile_pool(name="ps_cd", bufs=1, space="PSUM") as ps_cd:

            pending_out = None
            for j in (0, 2, 3, 1):
                ngrp = j + 1
                oa = [ps_oa.tile([128, 512], dt.float32, tag=f"oa{fc}", name=f"oa{fc}")
                      for fc in range(4)]
                wo_j = dict(wo_res)

                def fetch_wo(h):
                    if h not in wo_j and h < HEADS:
                        t = p_wos.tile([128, EMB], dt.bfloat16, tag="wos")
                        nc.gpsimd.dma_start(out=t, in_=woT[h * 128:(h + 1) * 128, :])
                        wo_j[h] = t

                def emit_scores(h, g, attns):
                    sT = ps_sc.tile([128, 512], dt.float32, tag="s", name="sT")
                    for s4 in range(4):
                        s = g * 4 + s4
                        nc.tensor.matmul(
                            sT[:, s4 * 128:(s4 + 1) * 128],
                            kT_sb[:, h * S + s * 128: h * S + (s + 1) * 128],
                            qT_sb[:, h * NQ + j * 128: h * NQ + (j + 1) * 128],
                            start=(s4 == 0), stop=(s4 == 3),
                            skip_group_check=True)
                    attn_t = p_attn.tile([128, 512], dt.bfloat16, tag="attn")
                    nc.scalar.activation(
                        attn_t, sT, mybir.ActivationFunctionType.Exp, scale=scale)
                    if g == j:
                        pending_masks.append(attn_t)
                    attns[(h, g)] = attn_t

                def emit_outproj(h, ctxT_h):
                    wo_h = wo_j[h]
                    for fc in range(4):
                        nc.tensor.matmul(
                            oa[fc],
                            ctxT_h,
                            wo_h[:, fc * 512:(fc + 1) * 512],
                            start=(h == 0), stop=(h == HEADS - 1),
                            skip_group_check=True)

                def emit_transpose(h):
                    # PE transpose + DVE copy of head h's normalized context.
                    # Deferred into head h+1's emission so the DVE
                    # recip/ts_mul chain has had time to complete.
                    trps = ps_sc.tile([128, 128], dt.bfloat16, tag="s", name="trps")
                    nc.tensor.transpose(trps, ctxns.pop(h), ident)
                    ctxT = p_ctxT.tile([128, 128], dt.bfloat16, tag="ctxT")
                    nc.vector.tensor_copy(ctxT, trps)
                    ctxTs[h] = ctxT

                def flush_masks():
                    while pending_masks:
                        attn_t = pending_masks.pop(0)
                        nc.vector.tensor_mul(
                            attn_t, attn_t, masks_sb[:, j * 512:(j + 1) * 512])

                def flush_out(oa_t, jj, last):
                    out_t = p_out.tile([128, EMB], dt.float32, tag="out_t",
                                       bufs=1)
                    for fc in range(4):
                        sl = out_t[:, fc * 512:(fc + 1) * 512]
                        if last and fc % 2 == 1:
                            nc.scalar.copy(sl, oa_t[fc])
                        else:
                            nc.vector.tensor_copy(sl, oa_t[fc])
                    nc.sync.dma_start(
                        out=out[jj * 128:(jj + 1) * 128, :], in_=out_t)

                ctxTs = {}
                ctxns = {}
                pending_masks = []
                # masked group first: its exp(+mask) chain is the longest.
                grp_order = [j] + list(range(j))
                # score-unit stream, pumped 2 units ahead of ctx consumption
                squeue = [(h, g) for h in range(HEADS) for g in grp_order]
                attns = {}
                si = 0

                def pump(n):
                    nonlocal si
                    for _ in range(n):
                        if si < len(squeue):
                            hh, gg = squeue[si]
                            emit_scores(hh, gg, attns)
                            si += 1

                pump(2)
                flush_masks()
                for h in range(HEADS):
                    # stream w_out for non-resident heads, ~4 ahead of use
                    if h + 4 < HEADS:
                        fetch_wo(h + 4)
                    if h >= 2:
                        emit_outproj(h - 2, ctxTs.pop(h - 2))
                    pump(1)
                    if h >= 1:
                        emit_transpose(h - 1)
                    pump(ngrp - 1)
                    if h == 1 and pending_out is not None:
                        flush_out(*pending_out, last=False)
                        pending_out = None
                    cd = ps_cd.tile([128, D + 1], dt.float32, tag="cd", name="cd")
                    for i, g in enumerate(grp_order):
                        attn_t = attns.pop((h, g))
                        for s4 in range(4):
                            s = g * 4 + s4
                            nc.tensor.matmul(
                                cd,
                                attn_t[:, s4 * 128:(s4 + 1) * 128],
                                v_sb[:, (s * HEADS + h) * (D + 1):
                                     (s * HEADS + h + 1) * (D + 1)],
                                start=(i == 0 and s4 == 0),
                                stop=(i == ngrp - 1 and s4 == 3),
                                skip_group_check=True)
                    rcp = p_rcp.tile([128, 1], dt.float32, tag="rcp")
                    nc.vector.reciprocal(rcp, cd[:, D:D + 1])
                    ctxn = p_ctxn.tile([128, 128], dt.bfloat16, tag="ctxn")
                    nc.vector.tensor_scalar_mul(ctxn, cd[:, 0:D], rcp)
                    ctxns[h] = ctxn
                    flush_masks()
                emit_outproj(HEADS - 2, ctxTs.pop(HEADS - 2))
                emit_transpose(HEADS - 1)
                emit_outproj(HEADS - 1, ctxTs.pop(HEADS - 1))
                pending_out = (oa, j)
            flush_out(*pending_out, last=True)
        wores_cm.__exit__(None, None, None)

    nc.finalize()
    return nc


def _shard_inputs(x, w_q, w_down, w_up_k, w_up_v, w_out, b_out):
    """Build the 8 per-core input maps (host-side layout prep)."""
    f32 = np.float32
    x = np.asarray(x, f32)
    wqT = np.ascontiguousarray(np.asarray(w_q, f32).T).astype(bf16)
    wq4 = np.ascontiguousarray(
        wqT.reshape(16, 128, 8, 256).transpose(2, 1, 0, 3).reshape(8, 128, 16 * 256))
    wdT = np.ascontiguousarray(np.asarray(w_down, f32).T).astype(bf16)
    wukT = np.ascontiguousarray(np.asarray(w_up_k, f32).T).astype(bf16)
    wuvT = np.ascontiguousarray(np.asarray(w_up_v, f32).T).astype(bf16)
    woT = np.ascontiguousarray(np.asarray(w_out, f32).T).astype(bf16)

    xTs = [np.ascontiguousarray(x[b].T).astype(bf16) for b in range(B)]

    in_maps = []
    for c in range(NCORES):
        b, idx = c // 4, c % 4
        gs = [idx + 4 * j for j in range(QB)]
        xT = xTs[b]
        xTq = np.ascontiguousarray(
            np.concatenate([xT[:, g * 128:(g + 1) * 128] for g in gs], axis=1))
        # masks[4, 128, 512]: only block j's final key-group can be masked
        m = np.zeros((NGRP, 128, 512), dtype=bf16)
        tri = (np.arange(128)[:, None] <= np.arange(128)[None, :]).astype(bf16)
        onem = np.ones((128, 128), dtype=bf16)
        for j in range(QB):
            g = gs[j]
            for s4 in range(4):
                s = 4 * j + s4
                if s < g:
                    m[j, :, s4 * 128:(s4 + 1) * 128] = onem
                elif s == g:
                    m[j, :, s4 * 128:(s4 + 1) * 128] = tri
        in_maps.append({
            "xT": xT, "xTq": xTq, "wdT": wdT, "wukT": wukT, "wuvT": wuvT,
            "wq4": wq4, "woT": woT, "masks": m,
        })
    return in_maps


def _unshard(results, dtype, b_out=None):
    out = np.zeros((B, S, EMB), dtype=np.float32)
    for c in range(NCORES):
        b, idx = c // 4, c % 4
        o = results[c]["out"]
        for j in range(QB):
            g = idx + 4 * j
            out[b, g * 128:(g + 1) * 128, :] = o[j * 128:(j + 1) * 128, :]
    if b_out is not None:
        out += np.asarray(b_out, np.float32)[None, None, :]
    return out.astype(dtype)


def kernel(x, w_q, w_down, w_up_k, w_up_v, w_out, b_out):
    from concourse.bass_utils import run_bass_kernel_spmd
    if "nc" not in _CACHE:
        _CACHE["nc"] = build_program()
    nc = _CACHE["nc"]
    in_maps = _shard_inputs(x, w_q, w_down, w_up_k, w_up_v, w_out, b_out)
    res = run_bass_kernel_spmd(nc, in_maps, list(range(NCORES)))
    return _unshard(res.results, np.asarray(x).dtype, b_out)


if __name__ == "__main__":
    import reference
    inputs = {k: np.asarray(v) for k, v in reference.setup_inputs().items()}
    got = kernel(**inputs)
    want = np.asarray(reference.reference(**inputs))
    err = np.abs(got - want)
    print("absmax rel err:", err.max() / np.abs(want).max())
